# revision 1
# baseline (speedup 1.0000x reference)
"""Trainium2 Bass kernel for CropAndResize (bilinear, TF semantics).

Strategy (8 NeuronCores, SPMD):
  - Shard by image: core k owns image k and the boxes routed to it by
    box_ind (padded to a uniform cap so all cores run one program).
  - On-chip: the core's image half (128 channels) lives in SBUF as fp16
    *adjacent pairs* (img_dup[c, k, :] = (v[k], v[k+1])), so one POOL
    ap_gather with d=2 fetches a bilinear (left, right) pair per index.
  - Per 8-box call: gather top/bottom pairs for all 196 crop positions,
    x-lerp via an interleaved (1-xl, xl) weight multiply + pair-sum,
    y-lerp via 3 DVE passes, write fp32 to DRAM.
  - Indices/weights are computed on device from `boxes`; out-of-range
    samples are redirected to a zeroed pad slot (mask fold), matching
    the reference's extrapolation_value=0.0 semantics.

Gather index wrapping: POOL consumes indices wrapped per 16-partition
group: output position j uses the index stored at partition j%16, slot
j//16. We choose flat output order (nh, i, jj, tb, nlo) with the inner
16 = (tb in {top,bottom}) x (nlo = box%8), so partition p owns a fixed
(tb, nlo) and the whole index tile is computed with partition-uniform
APs -- the tb selection enters via a per-partition scalar (+0/+1 row).
"""

import math

import numpy as np

import concourse.bass as bass
import concourse.bacc as bacc
import concourse.tile as tile
from concourse import mybir
from concourse.bass_utils import run_bass_kernel_spmd

B, C, H, W = 8, 256, 160, 160
CH, CW = 14, 14
HW = H * W  # 25600
N_CORES = 8
CK = 1600  # image-load chunk columns (fp32)

F32 = mybir.dt.float32
F16 = mybir.dt.float16
I16 = mybir.dt.int16

_PROGRAM_CACHE = {}


def _ap(base, extra_offset, pattern):
    return bass.AP(base.tensor, base.offset + extra_offset, pattern)


def build_program(cap):
    """Build the SPMD Bass program for `cap` boxes per core (cap % 8 == 0)."""
    capo = cap // 8
    nc = bacc.Bacc("TRN2", target_bir_lowering=False, debug=False)

    img_d = nc.dram_tensor("img", [257 * HW], F32, kind="ExternalInput")
    boxes_d = nc.dram_tensor("boxes", [cap, 4], F32, kind="ExternalInput")
    dy_d = nc.dram_tensor("dy", [128, 1], F32, kind="ExternalInput")
    iota14_d = nc.dram_tensor("iota14", [128, CH], F32, kind="ExternalInput")
    out_d = nc.dram_tensor("out", [cap, C, CH * CW], F32, kind="ExternalOutput")

    NE = HW + 1  # gather num_elems (zeroed pad pair at index HW)
    NI = 196 * 16  # num_idxs per call: (14*14) * (tb2 * nlo8)
    A = mybir.AluOpType
    INV13 = float(np.float32(1.0) / np.float32(CH - 1))
    EPS = 1e-3
    ADD, SUB, MUL, DIV, MOD = A.add, A.subtract, A.mult, A.divide, A.mod
    MAXO, MINO, GE, LE = A.max, A.min, A.is_ge, A.is_le

    with tile.TileContext(nc) as tc:
        with (
            tc.tile_pool(name="big", bufs=1) as bigp,
            tc.tile_pool(name="gch", bufs=2) as gchp,
            tc.tile_pool(name="one", bufs=1) as onep,
        ):
            # ---------------- constants ----------------
            dy = onep.tile([128, 1], F32, tag="dy")
            nc.sync.dma_start(dy[:], dy_d[:])
            iota14 = onep.tile([128, CH], F32, tag="iota")
            nc.sync.dma_start(iota14[:], iota14_d[:])

            # boxes striped so partition p = g*16 + tb*8 + nlo holds box
            # rows n = nh*8 + nlo
            bstr = onep.tile([128, capo, 4], F32, tag="bstr")
            src = boxes_d[:].rearrange("(nh nl) c -> nl nh c", nl=8)
            for g in range(8):
                for tb in range(2):
                    p0 = g * 16 + tb * 8
                    nc.sync.dma_start(bstr[p0 : p0 + 8, :, :], src)

            def coord_pipeline(pre, co_lo, co_hi, delta_ap):
                """mask [128,capo,14] and clamped floor idx [128,capo,14]."""
                hs = onep.tile([128, capo], F32, tag=f"{pre}hs")
                lo = bstr[:, :, co_lo]
                hi = bstr[:, :, co_hi]
                nc.vector.tensor_tensor(out=hs[:], in0=hi, in1=lo, op=SUB)
                nc.vector.tensor_scalar(hs[:], hs[:], float(H - 1), None, op0=MUL)
                nc.vector.tensor_scalar(hs[:], hs[:], INV13, None, op0=MUL)
                inv = onep.tile([128, capo, CH], F32, tag=f"{pre}inv")
                io_b = _ap(iota14[:], 0, [iota14[:].ap[0], [0, capo], [1, CH]])
                hs_b = _ap(hs[:], 0, [hs[:].ap[0], [1, capo], [0, CH]])
                nc.vector.tensor_tensor(out=inv[:], in0=io_b, in1=hs_b, op=MUL)
                lo_b = _ap(bstr[:], co_lo, [bstr[:].ap[0], [4, capo], [0, CH]])
                nc.vector.scalar_tensor_tensor(
                    inv[:], lo_b, float(H - 1), inv[:], op0=MUL, op1=ADD
                )
                m0 = onep.tile([128, capo, CH], F32, tag=f"{pre}m0")
                nc.vector.tensor_scalar(m0[:], inv[:], -EPS, None, op0=GE)
                m1 = onep.tile([128, capo, CH], F32, tag=f"{pre}m1")
                nc.vector.tensor_scalar(m1[:], inv[:], float(H - 1) + EPS, None, op0=LE)
                nc.vector.tensor_tensor(out=m0[:], in0=m0[:], in1=m1[:], op=MUL)
                # floor via int32 round-trip (mod is not ISA-encodable):
                # fl = cast(inv); fl -= (fl > inv)
                ii = onep.tile([128, capo, CH], mybir.dt.int32, tag=f"{pre}ii")
                nc.vector.tensor_copy(out=ii[:], in_=inv[:])
                nc.vector.tensor_copy(out=m1[:], in_=ii[:])
                gtm = onep.tile([128, capo, CH], F32, tag=f"{pre}ii")
                nc.vector.tensor_tensor(out=gtm[:], in0=m1[:], in1=inv[:], op=A.is_gt)
                nc.vector.tensor_tensor(out=inv[:], in0=m1[:], in1=gtm[:], op=SUB)
                if delta_ap is not None:
                    nc.vector.tensor_scalar(inv[:], inv[:], delta_ap, 0.0, op0=ADD, op1=MAXO)
                else:
                    nc.vector.tensor_scalar(inv[:], inv[:], 0.0, None, op0=MAXO)
                nc.vector.tensor_scalar(inv[:], inv[:], float(H - 1), None, op0=MINO)
                return m0, inv

            my, yi = coord_pipeline("y", 0, 2, dy[:, :1])
            mx, xi = coord_pipeline("x", 1, 3, None)

            # ---------------- wrapped gather indices ----------------
            widx = bigp.tile([128, capo * 196], I16, tag="widx")
            for nh in range(capo):
                ic = onep.tile([128, CH, CW], F32, tag="idxc")
                o14 = nh * CH
                yi_b = _ap(yi[:], o14, [yi[:].ap[0], [1, CH], [0, CW]])
                xi_b = _ap(xi[:], o14, [xi[:].ap[0], [0, CH], [1, CW]])
                nc.vector.scalar_tensor_tensor(ic[:], yi_b, float(W), xi_b, op0=MUL, op1=ADD)
                nc.vector.tensor_scalar(ic[:], ic[:], float(HW), None, op0=SUB)
                my_b = _ap(my[:], o14, [my[:].ap[0], [1, CH], [0, CW]])
                nc.vector.tensor_tensor(out=ic[:], in0=ic[:], in1=my_b, op=MUL)
                mx_b = _ap(mx[:], o14, [mx[:].ap[0], [0, CH], [1, CW]])
                nc.vector.tensor_tensor(out=ic[:], in0=ic[:], in1=mx_b, op=MUL)
                nc.vector.tensor_scalar(ic[:], ic[:], float(HW), None, op0=ADD)
                wslice = widx[:, nh * 196 : (nh + 1) * 196].rearrange(
                    "p (i j) -> p i j", i=CH
                )
                nc.vector.tensor_copy(out=wslice, in_=ic[:])

            # ---------------- weights (1-lane compute + broadcast) ----------
            box1 = onep.tile([1, cap * 4], F32, tag="box1")
            nc.sync.dma_start(box1[:], boxes_d[:].rearrange("n c -> (n c)").unsqueeze(0))

            one1 = onep.tile([1, 1], F32, tag="one1")
            nc.vector.memset(one1[:], 1.0)

            def lane_frac(co):
                """frac(in_v) on partition 0, laid out [1, n(=nh*8+nl), 14]."""
                hs1 = onep.tile([1, cap], F32, tag="hs1")
                hi = _ap(box1[:], co + 2, [box1[:].ap[0], [4, cap]])
                lo = _ap(box1[:], co, [box1[:].ap[0], [4, cap]])
                nc.vector.tensor_tensor(out=hs1[:], in0=hi, in1=lo, op=SUB)
                nc.vector.tensor_scalar(hs1[:], hs1[:], float(H - 1), None, op0=MUL)
                nc.vector.tensor_scalar(hs1[:], hs1[:], INV13, None, op0=MUL)
                fr1 = onep.tile([1, cap, CH], F32, tag="laneA")
                io_b = _ap(iota14[0:1, :], 0, [iota14[0:1, :].ap[0], [0, cap], [1, CH]])
                hs_b = _ap(hs1[:], 0, [hs1[:].ap[0], [1, cap], [0, CH]])
                nc.vector.tensor_tensor(out=fr1[:], in0=io_b, in1=hs_b, op=MUL)
                lo_b = _ap(box1[:], co, [box1[:].ap[0], [4, cap], [0, CH]])
                nc.vector.scalar_tensor_tensor(
                    fr1[:], lo_b, float(H - 1), fr1[:], op0=MUL, op1=ADD
                )
                ii1 = onep.tile([1, cap, CH], mybir.dt.int32, tag="laneI")
                nc.vector.tensor_copy(out=ii1[:], in_=fr1[:])
                fl1 = onep.tile([1, cap, CH], F32, tag="laneF")
                nc.vector.tensor_copy(out=fl1[:], in_=ii1[:])
                g1 = onep.tile([1, cap, CH], F32, tag="laneI")
                nc.vector.tensor_tensor(out=g1[:], in0=fl1[:], in1=fr1[:], op=A.is_gt)
                nc.vector.tensor_tensor(out=fl1[:], in0=fl1[:], in1=g1[:], op=SUB)
                nc.vector.tensor_tensor(out=fr1[:], in0=fr1[:], in1=fl1[:], op=SUB)
                return fr1

            # x first, then broadcast, then y reuses the same lane slots.
            # fr1 is [1, (nh nl), jj]; weight tiles need (nh, jj, nl) order,
            # so the copies below permute via 4-dim APs (TT ops only --
            # tensor_scalar is limited to 3-dim patterns by the verifier).
            fx1 = lane_frac(1)
            wx1 = onep.tile([1, capo, CW, 8, 2], F16, tag="laneB")
            fx_ap = _ap(fx1[:], 0, [fx1[:].ap[0], [8 * CH, capo], [1, CW], [CH, 8]])
            one_b = _ap(one1[:], 0, [one1[:].ap[0], [0, capo], [0, CW], [0, 8]])
            nc.vector.tensor_tensor(out=wx1[:, :, :, :, 1], in0=fx_ap, in1=one_b, op=MUL)
            nc.vector.tensor_tensor(out=wx1[:, :, :, :, 0], in0=one_b, in1=fx_ap, op=SUB)
            wx = bigp.tile([128, capo * CW * 16], F16, tag="wx")
            nc.gpsimd.partition_broadcast(
                wx[:], wx1[:].rearrange("p a b c d -> p (a b c d)")
            )
            fy1 = lane_frac(0)
            wy1 = onep.tile([1, capo, CH, 8], F16, tag="laneB")
            fy_ap = _ap(fy1[:], 0, [fy1[:].ap[0], [8 * CH, capo], [1, CH], [CH, 8]])
            one_b2 = _ap(one1[:], 0, [one1[:].ap[0], [0, capo], [0, CH], [0, 8]])
            nc.vector.tensor_tensor(out=wy1[:], in0=fy_ap, in1=one_b2, op=MUL)
            wy = bigp.tile([128, capo * CH * 8], F16, tag="wy")
            nc.gpsimd.partition_broadcast(wy[:], wy1[:].rearrange("p a b c -> p (a b c)"))

            # ---------------- main loop ----------------
            imgdup = bigp.tile([128, NE, 2], F16, tag="imgdup")
            nchunks = HW // CK
            for h in range(2):
                for k in range(nchunks):
                    ch = gchp.tile([128, CK + 1], F32, tag="gch")
                    off = (h * 128) * HW + k * CK
                    nc.sync.dma_start(ch[:], _ap(img_d[:], off, [[HW, 128], [1, CK + 1]]))
                    nc.scalar.copy(imgdup[:, k * CK : (k + 1) * CK, 0], ch[:, 0:CK])
                    nc.scalar.copy(imgdup[:, k * CK : (k + 1) * CK, 1], ch[:, 1 : CK + 1])
                nc.vector.memset(imgdup[:, HW : HW + 1, :], 0.0)

                for cc in range(capo):
                    gt = gchp.tile([128, 3136, 2], F16, tag="gch")
                    nc.gpsimd.ap_gather(
                        gt[:],
                        imgdup[:],
                        widx[:, cc * 196 : (cc + 1) * 196],
                        channels=128,
                        num_elems=NE,
                        d=2,
                        num_idxs=NI,
                    )
                    # x-lerp: multiply by interleaved (1-xl, xl), in place.
                    # gt free order: (i, jj, tb, nlo, lr); one op per tb so
                    # every AP stays within 3 free dims (ISA TENSOR3D limit).
                    wx_ap = _ap(
                        wx[:],
                        cc * CW * 16,
                        [wx[:].ap[0], [0, CH], [16, CW], [1, 16]],
                    )
                    for tb in range(2):
                        gt_tb = _ap(
                            gt[:],
                            tb * 16,
                            [gt[:].ap[0], [448, CH], [32, CW], [1, 16]],
                        )
                        nc.vector.tensor_tensor(out=gt_tb, in0=gt_tb, in1=wx_ap, op=MUL)
                    # pair-sum -> pp [128, (i j), tb, nlo]
                    pp = onep.tile([128, 196, 2, 8], F16, tag="pp")
                    l_v = gt[:, :, 0].rearrange("p (s t n) -> p s t n", t=2, n=8)
                    r_v = gt[:, :, 1].rearrange("p (s t n) -> p s t n", t=2, n=8)
                    nc.vector.tensor_tensor(out=pp[:], in0=l_v, in1=r_v, op=ADD)
                    # y-lerp
                    d2 = onep.tile([128, 196, 8], F16, tag="d2")
                    nc.vector.tensor_tensor(
                        out=d2[:], in0=pp[:, :, 1, :], in1=pp[:, :, 0, :], op=SUB
                    )
                    d2_v = d2[:].rearrange("p (i j) n -> p i j n", i=CH)
                    wy_ap = _ap(
                        wy[:], cc * CH * 8, [wy[:].ap[0], [8, CH], [0, CW], [1, 8]]
                    )
                    nc.vector.tensor_tensor(out=d2_v, in0=d2_v, in1=wy_ap, op=MUL)
                    stg = onep.tile([128, 8, 196], F32, tag=("laneA" if cc % 2 == 0 else "laneI"))
                    stg_ap = _ap(stg[:], 0, [stg[:].ap[0], [1, 196], [196, 8]])
                    nc.vector.tensor_tensor(
                        out=stg_ap, in0=pp[:, :, 0, :], in1=d2[:], op=ADD
                    )
                    dst = out_d[
                        cc * 8 : (cc + 1) * 8, h * 128 : (h + 1) * 128, :
                    ].rearrange("n c f -> c n f")
                    nc.sync.dma_start(dst, stg[:])

    nc.compile()
    return nc


def _host_consts():
    dy = np.zeros((128, 1), np.float32)
    for p in range(128):
        dy[p, 0] = float((p % 16) // 8)
    iota14 = np.broadcast_to(np.arange(CH, dtype=np.float32), (128, CH)).copy()
    return dy, iota14


def make_in_maps(image, boxes, box_ind):
    image = np.asarray(image, dtype=np.float32)
    boxes = np.asarray(boxes, dtype=np.float32)
    box_ind = np.asarray(box_ind, dtype=np.int32)

    order = np.argsort(box_ind, kind="stable")
    counts = np.bincount(box_ind, minlength=N_CORES)
    cap = max(8, int(math.ceil(counts.max() / 8.0)) * 8)
    starts = np.zeros(N_CORES + 1, np.int64)
    starts[1:] = np.cumsum(counts)

    dy, iota14 = _host_consts()
    in_maps = []
    for k in range(N_CORES):
        img_k = np.empty(257 * HW, np.float32)
        img_k[: 256 * HW] = image[k].reshape(-1)
        img_k[256 * HW :] = 0.0
        bk = np.zeros((cap, 4), np.float32)
        sel = order[starts[k] : starts[k + 1]]
        bk[: counts[k]] = boxes[sel]
        in_maps.append({"img": img_k, "boxes": bk, "dy": dy, "iota14": iota14})
    return in_maps, order, counts, starts, cap


def kernel(image, boxes, box_ind):
    in_maps, order, counts, starts, cap = make_in_maps(image, boxes, box_ind)

    nc = _PROGRAM_CACHE.get(cap)
    if nc is None:
        nc = build_program(cap)
        _PROGRAM_CACHE[cap] = nc

    res = run_bass_kernel_spmd(nc, in_maps, core_ids=list(range(N_CORES)))

    n = boxes.shape[0]
    out = np.empty((n, C, CH, CW), np.float32)
    for k in range(N_CORES):
        sel = order[starts[k] : starts[k + 1]]
        ok = res.results[k]["out"][: counts[k]].reshape(counts[k], C, CH, CW)
        out[sel] = ok
    return out

